# revision 2
# baseline (speedup 1.0000x reference)
"""EMA (first-order linear recurrence) kernel for Trainium2, 8 NeuronCores.

Problem: y[b, t, d] = a*y[b, t-1, d] + (1-a)*x[b, t, d],  y[b, -1, d] = 0,
x shape (4, 4096, 2048) f32, ALPHA = 0.99.

Strategy (all arithmetic in strict fp32 — matches the f32 reference to ~1e-6):
  - Shard d_model over the 8 cores (256 columns each). Each core handles
    x_sh (4, 4096, 256) independently; free-dim columns are (b, d) pairs,
    N = 4*256 = 1024 per core.
  - Chunked parallel scan over seq: 32 blocks of L=128 steps.
    Phase A per block j: DMA in X_j [128, 1024] (partition = t, free = (b,d)),
      TensorE matmul with the lower-triangular EMA matrix T (T[i,k] =
      (1-a)*a^(i-k)) gives the block-local scan Y_j = T @ X_j; ScalarE copies
      PSUM -> SBUF; the last row (w_j, the block-local scan end state) is
      moved to partition j of W via a tiny SBUF->SBUF DMA.
    Phase "scan" per group of 8 blocks: one small matmul C = S^T @ W computes
      the inter-block carries c_j = sum_{m<j} (a^L)^(j-1-m) w_m.
    Phase B per block j: replicate c_j across the 128 partitions with
      log-doubling SBUF->SBUF DMAs, then one fused VectorE op applies the
      carry fixup in place: E_j = (c_rep * P) + Y_j with P[i] = a^(i+1);
      DMA out.
  - Grouping (8 blocks/group) lets phase B of group g overlap phase A of
    group g+1, keeping the DMA engines (the roofline) busy end-to-end.

The walrus build in this container supports only ONE sync-wait per
instruction ("Too many sync wait commands" otherwise), while TileContext
emits several; _legalize_waits splits extras onto same-engine NOPs, which is
semantics-preserving because engines execute their streams in order.
"""
import numpy as np
from contextlib import ExitStack

ALPHA = 0.99
B, S, DM = 4, 4096, 2048
NCORES = 8
DS = DM // NCORES          # 256 d-columns per core
L = 128                    # seq block length
NB = S // L                # 32 blocks
N = B * DS                 # 1024 free columns per core
GRP = 8                    # blocks per carry-scan group
NH = 512                   # matmul moving-operand half (PSUM bank limit, fp32)

_cache = {}


# ---------------------------------------------------------------------------
# walrus wait-count legalization
# ---------------------------------------------------------------------------
def _legalize_waits(nc, max_waits=1):
    import concourse.mybir as mybir

    counter = [0]

    def split(blk):
        insts = blk.instructions
        i = 0
        while i < len(insts):
            inst = insts[i]
            for sub in (getattr(inst, "blocks", None) or []):
                split(sub)
            si = inst.sync_info
            if si is not None and si.on_wait and len(si.on_wait) > max_waits:
                waits = list(si.on_wait)
                keep = waits[len(waits) - max_waits:]
                overflow = waits[: len(waits) - max_waits]
                nops = []
                for j in range(0, len(overflow), max_waits):
                    chunk = overflow[j: j + max_waits]
                    counter[0] += 1
                    nop = mybir.InstNoOp(name=f"wsplit_nop_{counter[0]}")
                    nop.engine = inst.engine
                    nop.sync_info = mybir.SyncInfo(on_wait=chunk, on_update=[])
                    nops.append(nop)
                inst.sync_info = mybir.SyncInfo(
                    on_wait=keep, on_update=list(si.on_update)
                )
                for k, nop in enumerate(nops):
                    insts.insert(i + k, nop)
                i += len(nops)
            i += 1

    for fn in nc.m.functions:
        for blk in fn.blocks:
            split(blk)
    return nc


# ---------------------------------------------------------------------------
# constants
# ---------------------------------------------------------------------------
def _constants():
    a = float(ALPHA)
    ii = np.arange(L)
    diff = ii[None, :] - ii[:, None]              # i - k
    # T_T[k, i] = T[i, k] = (1-a) * a^(i-k) for k <= i else 0
    T_T = np.where(
        diff >= 0,
        (1.0 - a) * np.power(a, np.clip(diff, 0, None).astype(np.float64)),
        0.0,
    ).astype(np.float32)
    # S_T[m, j] = (a^L)^(j-1-m) for m <= j-1 else 0  (c_j = sum_m S_T[m,j] w_m)
    jj = np.arange(NB)
    djj = jj[None, :] - 1 - jj[:, None]
    aL = a ** L
    S_T = np.where(
        djj >= 0, np.power(aL, np.clip(djj, 0, None).astype(np.float64)), 0.0
    ).astype(np.float32)
    # P[i] = a^(i+1)
    P = np.power(a, (ii + 1).astype(np.float64)).astype(np.float32).reshape(L, 1)
    return T_T, S_T, P


def _build_nc():
    import concourse.bass as bass
    import concourse.tile as tile
    from concourse import mybir

    f32 = mybir.dt.float32
    AL = mybir.AluOpType
    T_T_np, S_T_np, P_np = _constants()

    nc = bass.Bass("TRN2", target_bir_lowering=False, debug=False)
    x = nc.dram_tensor("x_sh", [B, S, DS], f32, kind="ExternalInput")
    y = nc.dram_tensor("y_sh", [B, S, DS], f32, kind="ExternalOutput")
    tT_d = nc.inline_tensor(T_T_np, name="tT_const")
    sT_d = nc.inline_tensor(S_T_np, name="sT_const")
    p_d = nc.inline_tensor(P_np, name="p_const")

    with ExitStack() as ctx:
        tc = ctx.enter_context(tile.TileContext(nc))
        cpool = ctx.enter_context(tc.tile_pool(name="cpool", bufs=1))
        xpool = ctx.enter_context(tc.tile_pool(name="xpool", bufs=4))
        rpool = ctx.enter_context(tc.tile_pool(name="rpool", bufs=3))
        psum = ctx.enter_context(tc.tile_pool(name="psum", bufs=4, space="PSUM"))

        tT = cpool.tile([L, L], f32)
        nc.sync.dma_start(tT[:], tT_d.ap())
        sT = cpool.tile([NB, NB], f32)
        nc.sync.dma_start(sT[:], sT_d.ap())
        pcol = cpool.tile([L, 1], f32)
        nc.sync.dma_start(pcol[:], p_d.ap())

        Y = cpool.tile([L, NB * N], f32, tag="Y")       # resident local scans
        W = cpool.tile([NB, N], f32, tag="W")           # block end states
        C = cpool.tile([NB, N], f32, tag="C")           # inter-block carries

        xap, yap = x.ap(), y.ap()

        for g in range(NB // GRP):
            # ---------------- phase A for this group ----------------
            for j in range(g * GRP, (g + 1) * GRP):
                xt = xpool.tile([L, N], f32, tag="xt", name=f"xt{j}")
                for b in range(B):
                    nc.sync.dma_start(
                        xt[:, b * DS:(b + 1) * DS],
                        xap[b, j * L:(j + 1) * L, :],
                    )
                pt = psum.tile([L, N], f32, tag="mm", name=f"pt{j}")
                for h in range(N // NH):
                    nc.tensor.matmul(
                        pt[:, h * NH:(h + 1) * NH],
                        tT[:],
                        xt[:, h * NH:(h + 1) * NH],
                        start=True, stop=True,
                    )
                ysl = Y[:, j * N:(j + 1) * N]
                nc.scalar.copy(ysl, pt[:])
                # end state w_j -> partition j of W
                nc.sync.dma_start(W[j:j + 1, :], Y[127:128, j * N:(j + 1) * N])

            # ---------------- carry scan (rows 0 .. (g+1)*GRP-1) -----
            k = (g + 1) * GRP
            ct = psum.tile([NB, N], f32, tag="mm", name=f"ct{g}")
            for h in range(N // NH):
                nc.tensor.matmul(
                    ct[0:k, h * NH:(h + 1) * NH],
                    sT[0:k, 0:k],
                    W[0:k, h * NH:(h + 1) * NH],
                    start=True, stop=True,
                )
            # ScalarE requires 32-aligned partition bases; rows < g*GRP just
            # get rewritten with identical values.
            nc.scalar.copy(C[0:k, :], ct[0:k, :])

            # ---------------- phase B for this group ----------------
            for j in range(g * GRP, (g + 1) * GRP):
                ysl = Y[:, j * N:(j + 1) * N]
                if j > 0:
                    cr = rpool.tile([L, N], f32, tag="cr", name=f"cr{j}")
                    nc.sync.dma_start(cr[0:1, :], C[j:j + 1, :])
                    p = 1
                    while p < L:
                        nc.sync.dma_start(cr[p:2 * p, :], cr[0:p, :])
                        p *= 2
                    nc.vector.scalar_tensor_tensor(
                        ysl, cr[:], pcol[:], ysl, AL.mult, AL.add,
                    )
                for b in range(B):
                    nc.sync.dma_start(
                        yap[b, j * L:(j + 1) * L, :],
                        ysl[:, b * DS:(b + 1) * DS],
                    )
    return _legalize_waits(nc)


def _get_nc():
    if "nc" not in _cache:
        _cache["nc"] = _build_nc()
    return _cache["nc"]


def kernel(x) -> np.ndarray:
    from concourse.bass_utils import run_bass_kernel_spmd

    x = np.ascontiguousarray(np.asarray(x, dtype=np.float32))
    assert x.shape == (B, S, DM), x.shape
    nc = _get_nc()
    in_maps = [
        {"x_sh": np.ascontiguousarray(x[:, :, c * DS:(c + 1) * DS])}
        for c in range(NCORES)
    ]
    res = run_bass_kernel_spmd(nc, in_maps, core_ids=list(range(NCORES)))
    out = np.concatenate([res.results[c]["y_sh"] for c in range(NCORES)], axis=2)
    return out
